# revision 1
# baseline (speedup 1.0000x reference)
"""MiMo audio attention (B=2, S=2048, H=2048, NH=16, NKV=4, HD=128) on 8 trn2 cores.

Sharding: TP over heads x DP over batch. Cores 0-3 own batch 0, cores 4-7 own
batch 1. Within a batch group, TP rank t owns query heads [4t, 4t+4) and KV
head t (GQA: q head g uses kv head g//4, so the 4 q heads of rank t all use kv
head t). Each core computes a full-width o_proj partial over its 512
attn-output features; the host sums the 4 partials per batch group (the
"all-reduce after o_proj" of the TP scheme, done at gather time).

Device layout strategy (per core):
  - hidden is fed pre-transposed as hidT [H, S] so the QKV projections run
    with W as the stationary operand and produce Q^T/K^T/V^T [feat, tok].
  - RoPE is applied in the [feat, tok] layout: cos/sin tables [128, S] are
    host-precomputed; rotate_half becomes a 64-partition swap done with two
    SBUF->SBUF DMAs.
  - scores are computed transposed, S^T[k, q] = K^T_tile^T @ Q^T, so the
    softmax denominator is a ones-matmul (column sums) and attn@V needs no
    transposes: out^T[d, q] = V_tile^T @ exp(S^T).
  - softmax uses no max-subtraction (scores are O(5) for this distribution;
    exp is safe in fp32) and the causal mask is a multiplicative triangle
    applied only to diagonal tiles, post-exp.
  - matmuls run in float32r (tf32-like fast path, 4x over plain fp32).
"""

import numpy as np

import concourse.bass as bass
import concourse.mybir as mybir
import concourse.tile as tile
from concourse import bacc, bass_utils
from concourse.tile_rust import add_dep_helper

B, S, H = 2, 2048, 2048
NH, NKV, HD = 16, 4, 128
THETA = 10000.0
SCALE = HD ** -0.5

NCORES = 8
TP = 4                 # cores per batch group
HPC = NH // TP         # 4 query heads per core
KT = H // 128          # 16 contraction tiles for projections
TT = S // 512          # 4 token tiles of 512
ST = S // 128          # 16 token tiles of 128

F32 = mybir.dt.float32
F32R = mybir.dt.float32r
AF = mybir.ActivationFunctionType

_PROGRAM_CACHE = {}


def build_program(npasses=1):
    key = ("nc", npasses)
    if key in _PROGRAM_CACHE:
        return _PROGRAM_CACHE[key]

    nc = bacc.Bacc("TRN2", target_bir_lowering=False, debug=False, num_devices=NCORES)

    hidT = nc.declare_dram_parameter("hidT", [H, S], F32, isOutput=False)
    wq = nc.declare_dram_parameter("wq", [H, HPC * HD], F32, isOutput=False)
    wk = nc.declare_dram_parameter("wk", [H, HD], F32, isOutput=False)
    wv = nc.declare_dram_parameter("wv", [H, HD], F32, isOutput=False)
    wo = nc.declare_dram_parameter("wo", [HPC * HD, H], F32, isOutput=False)
    bq = nc.declare_dram_parameter("bq", [HD, HPC], F32, isOutput=False)
    bk = nc.declare_dram_parameter("bk", [HD, 1], F32, isOutput=False)
    bv = nc.declare_dram_parameter("bv", [HD, 1], F32, isOutput=False)
    cosT = nc.declare_dram_parameter("cosT", [HD, S], F32, isOutput=False)
    sinT = nc.declare_dram_parameter("sinT", [HD, S], F32, isOutput=False)
    mask = nc.declare_dram_parameter("mask", [128, 256], F32, isOutput=False)
    ones = nc.declare_dram_parameter("ones", [128, 128], F32, isOutput=False)
    eye = nc.declare_dram_parameter("eye", [128, 128], F32, isOutput=False)
    out_d = nc.declare_dram_parameter("out", [S, H], F32, isOutput=True)

    hidT_r = hidT.ap().rearrange("(a p) m -> p a m", p=128)   # [128, 16, 2048]
    wq_r = wq.ap().rearrange("(a p) m -> p a m", p=128)
    wk_r = wk.ap().rearrange("(a p) m -> p a m", p=128)
    wv_r = wv.ap().rearrange("(a p) m -> p a m", p=128)

    with tile.TileContext(nc) as tc:
        with (
            tc.tile_pool(name="consts", bufs=1) as consts,
            tc.tile_pool(name="persist", bufs=1) as persist,
            tc.tile_pool(name="vtr", bufs=2) as vtrp,
            tc.tile_pool(name="expp", bufs=4) as expp,
            tc.tile_pool(name="recp", bufs=2) as recp,
            tc.tile_pool(name="trig", bufs=1) as trig,
            tc.tile_pool(name="stage", bufs=3) as stage,
        ):
            mask_sb = consts.tile([128, 256], F32R)
            ones_sb = consts.tile([128, 128], F32R)
            eye_sb = consts.tile([128, 128], F32)
            bq_sb = consts.tile([HD, HPC], F32)
            bk_sb = consts.tile([HD, 1], F32)
            bv_sb = consts.tile([HD, 1], F32)
            nc.scalar.dma_start(bq_sb[:], bq.ap())
            nc.scalar.dma_start(bk_sb[:], bk.ap())
            nc.scalar.dma_start(bv_sb[:], bv.ap())
            nc.scalar.dma_start(eye_sb[:], eye.ap())

            def emit(pid):
                # persistent activations, one tile per (tensor, tok-tile) so
                # cross-phase dependencies stay precise
                qt_sb = [[persist.tile([128, 512], F32R, name=f"qt{h}_{t}", tag=f"qt{h}_{t}")
                          for t in range(TT)] for h in range(HPC)]
                kt_sb = [persist.tile([128, 512], F32R, name=f"kt_{t}", tag=f"kt_{t}") for t in range(TT)]
                v_sb = [persist.tile([128, 128], F32R, name=f"v_{i}", tag=f"v_{i}") for i in range(ST)]
                ao_sb = [[persist.tile([128, 512], F32R, name=f"ao{h}_{t}", tag=f"ao{h}_{t}")
                          for t in range(TT)] for h in range(HPC)]

                cos_sb = trig.tile([HD, S], F32, name="cos_sb")
                sin_sb = trig.tile([HD, S], F32, name="sin_sb")

                def rope_inplace(t, dst, after=None):
                    """dst holds raw (biased) values for tok tile t; rotate in place.

                    `after`: optional instruction; adds a scheduler-only edge so
                    this chain is ordered behind it (keeps the in-order DVE FIFO
                    from blocking earlier-needed work behind this chain)."""
                    tok = bass.ds(t * 512, 512)
                    swp = stage.tile([128, 512], F32, tag="swp")
                    d0 = nc.gpsimd.dma_start(swp[0:64, :], dst.bitcast(F32)[64:128, :])
                    d1 = nc.gpsimd.dma_start(swp[64:128, :], dst.bitcast(F32)[0:64, :])
                    if after is not None:
                        add_dep_helper(d0.ins, after.ins, False, "delay last-tok rope")
                        add_dep_helper(d1.ins, after.ins, False, "delay last-tok rope")
                    m0 = nc.vector.tensor_mul(dst[:], dst[:], cos_sb[:, tok])
                    if after is not None:
                        add_dep_helper(m0.ins, after.ins, False, "delay last-tok rope")
                    nc.vector.tensor_mul(swp[:], swp[:], sin_sb[:, tok])
                    nc.vector.tensor_add(dst[:], dst[:], swp[:])

                # ---------------- phase 1: QKV projection + RoPE -----------------
                with (
                    tc.tile_pool(name=f"wts{pid}", bufs=1) as wts,
                    tc.tile_pool(name=f"hidp{pid}", bufs=2) as hidp,
                    tc.tile_pool(name=f"ppsA{pid}", bufs=1, space=bass.MemorySpace.PSUM) as pps,
                    tc.tile_pool(name=f"ppsB{pid}", bufs=1, space=bass.MemorySpace.PSUM) as ppsB,
                ):
                    wq_sb = wts.tile([128, KT, HPC * HD], F32R)
                    wk_sb = wts.tile([128, KT, HD], F32R)
                    wv_sb = wts.tile([128, KT, HD], F32R)

                    vtr_tiles = {}
                    for t in range(TT):
                        tok = bass.ds(t * 512, 512)
                        q_ps = [pps.tile([128, 512], F32, name=f"qps{f}", tag=f"qps{f}") for f in range(HPC)]
                        k_ps = ppsB.tile([128, 512], F32, tag="kps")
                        v_ps = ppsB.tile([128, 512], F32, tag="vps")
                        # V^T -> V transposes for the PREVIOUS tok tile, emitted
                        # first (their inputs are long ready; copies go to ACT so
                        # they never queue behind RoPE work on the DVE)
                        if t > 0:
                            for i in range(4 * (t - 1), 4 * t):
                                tp = ppsB.tile([128, 128], F32, tag="vt", bufs=2)
                                nc.tensor.transpose(tp[:], vtr_tiles[t - 1][:, (i % 4) * 128:(i % 4 + 1) * 128], eye_sb[:])
                                nc.scalar.activation(v_sb[i][:], tp[:], AF.Identity)
                        for kc in range(KT // 4):       # 4 k-slices per DMA chunk
                            if t == 0:
                                nc.scalar.dma_start(wk_sb[:, 4 * kc:4 * (kc + 1), :],
                                                    wk_r[:, 4 * kc:4 * (kc + 1), :].bitcast(F32R))
                                nc.scalar.dma_start(wv_sb[:, 4 * kc:4 * (kc + 1), :],
                                                    wv_r[:, 4 * kc:4 * (kc + 1), :].bitcast(F32R))
                                nc.scalar.dma_start(wq_sb[:, 4 * kc:4 * (kc + 1), :],
                                                    wq_r[:, 4 * kc:4 * (kc + 1), :].bitcast(F32R))
                            ht = hidp.tile([128, 4, 512], F32R)
                            nc.sync.dma_start(ht[:], hidT_r[:, 4 * kc:4 * (kc + 1), tok].bitcast(F32R))
                            for kk in range(4):
                                k = 4 * kc + kk
                                st, sp = (k == 0), (k == KT - 1)
                                nc.tensor.matmul(k_ps[:], wk_sb[:, k, :], ht[:, kk, :], start=st, stop=sp)
                                for f in range(HPC):
                                    nc.tensor.matmul(q_ps[f][:], wq_sb[:, k, f * 128:(f + 1) * 128], ht[:, kk, :], start=st, stop=sp)
                                nc.tensor.matmul(v_ps[:], wv_sb[:, k, :], ht[:, kk, :], start=st, stop=sp)
                        if t == 0:
                            nc.scalar.dma_start(cos_sb[:], cosT.ap())
                            nc.scalar.dma_start(sin_sb[:], sinT.ap())

                        # pass 1: evacuate all six PSUM banks (alternating engines)
                        for f in range(HPC):
                            if f % 2 == 0:
                                nc.scalar.activation(qt_sb[f][t][:], q_ps[f][:], AF.Identity, bias=bq_sb[:, f:f + 1])
                            else:
                                nc.vector.tensor_scalar_add(qt_sb[f][t][:], q_ps[f][:], bq_sb[:, f:f + 1])
                        vtr = vtrp.tile([128, 512], F32, tag="vtr")
                        vtr_tiles[t] = vtr
                        nc.scalar.activation(vtr[:], v_ps[:], AF.Identity, bias=bv_sb[:])
                        nc.vector.tensor_scalar_add(kt_sb[t][:], k_ps[:], bk_sb[:])

                        # pass 2: RoPE in place on Q heads and K
                        if t < TT - 1:  # last tok tile RoPE is emitted in the attention block
                            for f in range(HPC):
                                rope_inplace(t, qt_sb[f][t])
                            rope_inplace(t, kt_sb[t])

                # ---------------- phase 2: attention + o_proj, j-outer -----------
                with (
                    tc.tile_pool(name=f"wo_p{pid}", bufs=1) as wo_p,
                    tc.tile_pool(name=f"outp{pid}", bufs=3) as outp,
                    tc.tile_pool(name=f"scps{pid}", bufs=2, space=bass.MemorySpace.PSUM) as scps,
                    tc.tile_pool(name=f"oups{pid}", bufs=2, space=bass.MemorySpace.PSUM) as oups,
                    tc.tile_pool(name=f"smps{pid}", bufs=2, space=bass.MemorySpace.PSUM) as smps,
                    tc.tile_pool(name=f"opps{pid}", bufs=2, space=bass.MemorySpace.PSUM) as opps,
                ):
                    wo_sb = wo_p.tile([128, HPC, H], F32R)
                    nc.scalar.dma_start(mask_sb[:], mask.ap().bitcast(F32R))
                    nc.scalar.dma_start(ones_sb[:], ones.ap().bitcast(F32R))

                    def load_wo_chunk(k):
                        nc.scalar.dma_start(wo_sb[:, k, :],
                                            wo.ap().rearrange("(t p) m -> p t m", p=128)[:, k, :].bitcast(F32R))

                    def attn_tile(h, j):
                        ou_ps = oups.tile([128, 512], F32, tag="ou")
                        sm_ps = smps.tile([128, 512], F32, tag="sm")
                        last = 4 * j + 3
                        pend = None  # software-pipeline: consumer MMs trail by one i
                        for i in range(last + 1):
                            d = i - 4 * j
                            c0 = 0 if d < 0 else min(128 * d, 256)
                            sc_ps = scps.tile([128, 512], F32, tag="sc")
                            nc.tensor.matmul(
                                sc_ps[:, c0:512],
                                kt_sb[i // 4][:, (i % 4) * 128:(i % 4 + 1) * 128],
                                qt_sb[h][j][:, c0:512],
                                start=True, stop=True,
                            )
                            ex = expp.tile([128, 512], F32R)
                            nc.scalar.activation(ex[:, c0:512], sc_ps[:, c0:512], AF.Exp, scale=SCALE)
                            if d >= 0:
                                delta = 128 * d
                                nc.vector.tensor_mul(
                                    ex[:, c0:delta + 128],
                                    ex[:, c0:delta + 128],
                                    mask_sb[:, c0 - delta + 128:256],
                                )
                            if pend is not None:
                                pex, pc0, pi = pend
                                nc.tensor.matmul(ou_ps[:, pc0:512], v_sb[pi][:], pex[:, pc0:512],
                                                 start=(pi == 0), stop=False)
                                nc.tensor.matmul(sm_ps[:, pc0:512], ones_sb[:], pex[:, pc0:512],
                                                 start=(pi == 0), stop=False)
                            pend = (ex, c0, i)
                        pex, pc0, pi = pend
                        nc.tensor.matmul(ou_ps[:, pc0:512], v_sb[pi][:], pex[:, pc0:512],
                                         start=(pi == 0), stop=True)
                        nc.tensor.matmul(sm_ps[:, pc0:512], ones_sb[:], pex[:, pc0:512],
                                         start=(pi == 0), stop=True)
                        rec = recp.tile([128, 512], F32)
                        nc.vector.reciprocal_approx_fast(rec[:], sm_ps[:])
                        return nc.vector.tensor_mul(ao_sb[h][j][:], ou_ps[:], rec[:])

                    def oproj_strip(j):
                        for m in range(4 * j, 4 * j + 4):
                            split_out = (m == 3)
                            ot = outp.tile([128, H], F32)
                            for n in range(TT):
                                ps = opps.tile([128, 512], F32, tag="op")
                                for k in range(HPC):
                                    nc.tensor.matmul(
                                        ps[:],
                                        ao_sb[k][m // 4][:, (m % 4) * 128:(m % 4 + 1) * 128],
                                        wo_sb[:, k, n * 512:(n + 1) * 512],
                                        start=(k == 0), stop=(k == HPC - 1),
                                    )
                                if n % 2 == 0:
                                    nc.scalar.activation(ot[:, n * 512:(n + 1) * 512], ps[:], AF.Identity)
                                else:
                                    nc.vector.tensor_copy(ot[:, n * 512:(n + 1) * 512], ps[:])
                                if split_out:
                                    nc.sync.dma_start(out_d.ap()[m * 128:(m + 1) * 128, n * 512:(n + 1) * 512],
                                                      ot[:, n * 512:(n + 1) * 512])
                            if not split_out:
                                nc.sync.dma_start(out_d.ap()[m * 128:(m + 1) * 128, :], ot[:])

                    # j=0 (all-diagonal, DVE-dependent) goes LAST so attention
                    # start never waits on the final tok tile's RoPE/DVE chain
                    for h in range(HPC):
                        attn_tile(h, 1)
                    for h in range(HPC):
                        a = attn_tile(h, 2)
                        load_wo_chunk(h)
                        # last tok tile's RoPE, spread between the j=2 strips so
                        # the in-order DVE never blocks j1/j2 mask work on it
                        rope_inplace(TT - 1, qt_sb[h][TT - 1], after=a)
                        if h == HPC - 1:
                            rope_inplace(TT - 1, kt_sb[TT - 1], after=a)
                    # last tok tile's V transposes (needed from attn j=3 on)
                    for i in range(4 * (TT - 1), ST):
                        tp = opps.tile([128, 128], F32, tag="op")
                        nc.tensor.transpose(tp[:], vtr_tiles[TT - 1][:, (i % 4) * 128:(i % 4 + 1) * 128], eye_sb[:])
                        nc.scalar.activation(v_sb[i][:], tp[:], AF.Identity)
                    oproj_strip(1)
                    for h in range(HPC):
                        attn_tile(h, 3)
                    oproj_strip(2)
                    for h in range(HPC):
                        attn_tile(h, 0)
                    oproj_strip(3)
                    oproj_strip(0)


            for pid in range(npasses):
                if pid > 0:
                    tc.strict_bb_all_engine_barrier()
                emit(pid)

    nc.compile()
    _PROGRAM_CACHE[key] = nc
    return nc


def build_in_maps(hidden_states, positions, Wq, bq, Wk, bk, Wv, bv, Wo):
    hidden_states = np.asarray(hidden_states, dtype=np.float32)
    positions = np.asarray(positions)
    Wq = np.asarray(Wq, dtype=np.float32)
    Wk = np.asarray(Wk, dtype=np.float32)
    Wv = np.asarray(Wv, dtype=np.float32)
    Wo = np.asarray(Wo, dtype=np.float32)
    bq = np.asarray(bq, dtype=np.float32)
    bk = np.asarray(bk, dtype=np.float32)
    bv = np.asarray(bv, dtype=np.float32)

    inv_freq = (1.0 / (THETA ** (np.arange(0, HD, 2, dtype=np.float32) / HD))).astype(np.float32)
    freqs = positions.astype(np.float32)[:, None] * inv_freq[None, :]      # [S, 64]
    cos_h = np.cos(freqs).T.astype(np.float32)                              # [64, S]
    sin_h = np.sin(freqs).T.astype(np.float32)
    cosT = np.ascontiguousarray(np.concatenate([cos_h, cos_h], axis=0))     # [128, S]
    sinT = np.ascontiguousarray(np.concatenate([-sin_h, sin_h], axis=0))    # [128, S]

    r = np.arange(128)[:, None]
    c = np.arange(256)[None, :]
    mask = (c >= r + 128).astype(np.float32)
    ones = np.ones((128, 128), dtype=np.float32)
    eye = np.eye(128, dtype=np.float32)

    hidT = [np.ascontiguousarray(hidden_states[g].T) for g in range(B)]

    in_maps = []
    for core in range(NCORES):
        g, t = core // TP, core % TP
        fs = slice(512 * t, 512 * (t + 1))
        ks = slice(128 * t, 128 * (t + 1))
        in_maps.append({
            "hidT": hidT[g],
            "wq": np.ascontiguousarray(Wq[:, fs]),
            "wk": np.ascontiguousarray(Wk[:, ks]),
            "wv": np.ascontiguousarray(Wv[:, ks]),
            "wo": np.ascontiguousarray(Wo[fs, :]),
            "bq": np.ascontiguousarray(bq[fs].reshape(HPC, HD).T),
            "bk": np.ascontiguousarray(bk[ks].reshape(HD, 1)),
            "bv": np.ascontiguousarray(bv[ks].reshape(HD, 1)),
            "cosT": cosT,
            "sinT": sinT,
            "mask": mask,
            "ones": ones,
            "eye": eye,
        })
    return in_maps


def assemble(results):
    out = np.empty((B, S, H), dtype=np.float32)
    for g in range(B):
        acc = results[TP * g]["out"].astype(np.float32).copy()
        for t in range(1, TP):
            acc += results[TP * g + t]["out"]
        out[g] = acc
    return out


def kernel(**inputs) -> np.ndarray:
    nc = build_program()
    in_maps = build_in_maps(**inputs)
    res = bass_utils.run_bass_kernel_spmd(nc, in_maps, list(range(NCORES)))
    return assemble(res.results)

